# revision 1
# baseline (speedup 1.0000x reference)
"""BiMamba Trainium2 kernel.

Sharding: each of the 8 cores owns a 256-channel slice of d_inner for BOTH
directions (fwd+rev share in_proj/out_proj, so the reversed direction's
in_proj output is just a flipped view of the forward one).  Per core:
  - in_proj:  xz[:, slice] = hidden @ W_in[slice].T          (PE, fp32r)
  - conv+silu: fwd reads the padded x buffer normally, rev reads it through
    reversed APs (zero pad on both ends)                     (ACT + DVE)
  - x_proj:   partial x_dbl summed over cores via AllReduce  (PE + collective)
  - dt_proj + softplus                                       (PE + ACT)
  - selective scan per (dir, ptile, n):  dA = exp(A_n * dt)  (ACT)
        dBx = (dt*x) * B_n                                   (DVE bf16)
        h   = tensor_tensor_scan(dA, dBx)                    (DVE/Pool)
        hC  = h * C_n ; y_ssm = sum_n hC + D*x (pair tree)   (DVE bf16)
  - gate: y = y_ssm * silu(z)                                (DVE/ACT)
  - out_proj partial -> ReduceScatter over token blocks      (PE + collective)
"""

import os
import sys

sys.path.insert(0, "/opt/trn_rl_repo")

import numpy as np
import ml_dtypes

# ---------------------------------------------------------------- constants
P = 128           # partitions
L = 2048          # sequence length
DM = 1024         # d_model
DI = 2048         # d_inner
NST = 16          # d_state
RK = 64           # dt_rank
KCONV = 4         # conv width
NCORES = 8
CH = DI // NCORES          # channels per core per direction = 256
NPT = CH // P              # channel ptiles per core = 2
FB = 512                   # matmul moving free chunk (fp32)
NFB = L // FB              # 4
PAD = KCONV - 1            # causal pad = 3
NXP = RK + 2 * NST         # 96
HCCH = 1024                # t-chunk for the hC/tree stage
NHC = L // HCCH            # 2


def build_program(num_cores=NCORES, scan_pool_frac=0.5, enable_asserts=False,
                  skip_scan=False):
    """Build the SPMD Bass program (same NEFF on every core)."""
    import concourse.bass as bass
    import concourse.mybir as mybir
    import concourse.tile as tile
    from concourse import bacc
    from contextlib import ExitStack

    dt = mybir.dt
    AF = mybir.ActivationFunctionType
    OP = mybir.AluOpType

    nc = bacc.Bacc(
        "TRN2",
        target_bir_lowering=False,
        debug=False,
        enable_asserts=enable_asserts,
        num_devices=num_cores,
    )

    # ------------------------------------------------------------- dram I/O
    hidden = nc.dram_tensor("hidden", [L, DM], dt.float32, kind="ExternalInput")
    w_inT = nc.dram_tensor("w_inT", [DM, 2 * CH], dt.bfloat16, kind="ExternalInput")
    w_outT = nc.dram_tensor("w_outT", [CH, DM], dt.bfloat16, kind="ExternalInput")
    w_xT = {}
    w_dtT = {}
    conv_w = {}
    conv_b = {}
    dt_b = {}
    A_in = {}
    D_in = {}
    for d in ("f", "r"):
        w_xT[d] = nc.dram_tensor(f"w_xT_{d}", [CH, NXP], dt.bfloat16,
                                 kind="ExternalInput")
        w_dtT[d] = nc.dram_tensor(f"w_dtT_{d}", [RK, CH], dt.float32,
                                  kind="ExternalInput")
        conv_w[d] = nc.dram_tensor(f"conv_w_{d}", [CH, KCONV], dt.float32,
                                   kind="ExternalInput")
        conv_b[d] = nc.dram_tensor(f"conv_b_{d}", [CH, 1], dt.float32,
                                   kind="ExternalInput")
        dt_b[d] = nc.dram_tensor(f"dt_b_{d}", [CH, 1], dt.float32,
                                 kind="ExternalInput")
        A_in[d] = nc.dram_tensor(f"A_{d}", [CH, NST], dt.float32,
                                 kind="ExternalInput")
        D_in[d] = nc.dram_tensor(f"D_{d}", [CH, 1], dt.float32,
                                 kind="ExternalInput")
    ident = nc.dram_tensor("ident", [P, P], dt.float32, kind="ExternalInput")
    out = nc.dram_tensor("out", [L // num_cores, DM], dt.float32,
                         kind="ExternalOutput")

    f32r = dt.float32r
    NKB = DM // P  # 8
    NTT = L // P   # 16
    WPAD = L + 2 * PAD  # padded x width (zeros both ends for the rev conv)

    with tile.TileContext(nc) as tc:
        ctx = ExitStack()
        with ctx:
            dram = ctx.enter_context(tc.tile_pool(name="dram", bufs=1, space="DRAM"))
            consts = ctx.enter_context(tc.tile_pool(name="consts", bufs=1))
            psum_mm = ctx.enter_context(
                tc.tile_pool(name="psum_mm", bufs=3, space="PSUM"))


            # ---------------------------------------------------- constants
            ident_sb = consts.tile([P, P], dt.float32)
            nc.sync.dma_start(ident_sb[:], ident[:])
            conv_w_sb = {}
            conv_b_sb = {}
            dt_b_sb = {}
            A_sb = {}
            D_sb = {}
            for d in ("f", "r"):
                for pb in range(NPT):
                    ps = slice(pb * P, (pb + 1) * P)
                    for nm, store, src, shape in (
                        ("cw", conv_w_sb, conv_w, [P, KCONV]),
                        ("cb", conv_b_sb, conv_b, [P, 1]),
                        ("db", dt_b_sb, dt_b, [P, 1]),
                        ("A", A_sb, A_in, [P, NST]),
                        ("Dc", D_sb, D_in, [P, 1]),
                    ):
                        t = consts.tile(shape, dt.float32, name=f"{nm}{d}{pb}",
                                        tag=f"{nm}{d}{pb}")
                        nc.sync.dma_start(t[:], src[d][ps, :])
                        store[d, pb] = t
            w_dt_sb = {}
            for d in ("f", "r"):
                w_dt_sb[d] = consts.tile([RK, CH], dt.float32, name=f"wdt{d}",
                                         tag=f"wdt{d}")
                nc.sync.dma_start(w_dt_sb[d][:], w_dtT[d][:])
            w_x_sb = {}
            for d in ("f", "r"):
                for pb in range(NPT):
                    t = consts.tile([P, NXP], dt.bfloat16, name=f"wx{d}{pb}",
                                    tag=f"wx{d}{pb}")
                    nc.sync.dma_start(t[:], w_xT[d][pb * P:(pb + 1) * P, :])
                    w_x_sb[d, pb] = t
            w_out_sb = []
            for pb in range(NPT):
                t = consts.tile([P, DM], dt.bfloat16, name=f"wo{pb}", tag=f"wo{pb}")
                nc.sync.dma_start(t[:], w_outT[pb * P:(pb + 1) * P, :])
                w_out_sb.append(t)

            # persistent activation buffers (gated-z, silu applied eagerly)
            gz_pool = ctx.enter_context(tc.tile_pool(name="gzp", bufs=1))
            gz = {}
            for d in ("f", "r"):
                for pb in range(NPT):
                    gz[d, pb] = gz_pool.tile([P, L], dt.bfloat16,
                                             name=f"gz{d}{pb}", tag=f"gz{d}{pb}")
            xc_pool = ctx.enter_context(tc.tile_pool(name="xcp", bufs=4))
            yc_pool = ctx.enter_context(tc.tile_pool(name="ycp", bufs=4))
            oev_pool = ctx.enter_context(tc.tile_pool(name="oevp", bufs=2))

            xdbl_part = dram.tile([2 * NXP, L], dt.float32)
            dram_bc16 = {}
            for d in ("f", "r"):
                dram_bc16[d] = dram.tile([2 * NST, L], dt.bfloat16,
                                         name=f"dbc16{d}", tag=f"dbc16{d}")
            xdbl_sum = dram.tile([2 * NXP, L], dt.float32, addr_space="Shared")
            pout = dram.tile([L, DM], dt.float32)
            pout_rs = dram.tile([L // num_cores, DM], dt.float32)

            # stage-limited pools (freed once early phases are emitted)
            ctxB = ExitStack()
            xpad_pool = ctxB.enter_context(tc.tile_pool(name="xpadp", bufs=1))
            cacc_pool = ctxB.enter_context(tc.tile_pool(name="caccp", bufs=3))
            xev_pool = ctxB.enter_context(tc.tile_pool(name="xevp", bufs=2))
            ctxA = ExitStack()
            hT_pool = ctxA.enter_context(tc.tile_pool(name="hTp", bufs=1))
            hnat_pool = ctxA.enter_context(tc.tile_pool(name="hnatp", bufs=8))
            w_in_pool = ctxA.enter_context(tc.tile_pool(name="winp", bufs=1))

            # ------------------------------------------- stage 1: hT = hidden^T
            psum_tp = ctxA.enter_context(
                tc.tile_pool(name="psum_tp", bufs=3, space="PSUM"))
            hT = [hT_pool.tile([P, L], dt.bfloat16, name=f"hT{k}", tag=f"hT{k}")
                  for k in range(NKB)]
            for q in range(NTT // 4):
                hn = []
                for j in range(4):
                    t = hnat_pool.tile([P, DM], dt.float32, name="hnat", tag="hnat")
                    nc.sync.dma_start(
                        t[:], hidden[(q * 4 + j) * P:(q * 4 + j + 1) * P, :])
                    hn.append(t)
                for kb in range(NKB):
                    pt = psum_tp.tile([P, 4 * P], dt.float32, name="tp", tag="tp")
                    for j in range(4):
                        nc.tensor.transpose(
                            pt[:, j * P:(j + 1) * P],
                            hn[j][:, kb * P:(kb + 1) * P],
                            ident_sb[:],
                        )
                    nc.scalar.copy(hT[kb][:, q * 4 * P:(q + 1) * 4 * P], pt[:])

            # ------------------------------------------- stage 2: in_proj
            w_in_sb = [w_in_pool.tile([P, 2 * CH], dt.bfloat16, name=f"win{k}",
                                      tag=f"win{k}") for k in range(NKB)]
            for k in range(NKB):
                nc.sync.dma_start(w_in_sb[k][:], w_inT[k * P:(k + 1) * P, :])

            xpad = [xpad_pool.tile([P, WPAD], dt.bfloat16, name=f"xpad{pb}",
                                   tag=f"xpad{pb}") for pb in range(NPT)]
            for pb in range(NPT):
                nc.vector.memset(xpad[pb][:, 0:PAD], 0.0)
                nc.vector.memset(xpad[pb][:, PAD + L:WPAD], 0.0)

            for mb in range(2 * NPT):
                for fb in range(NFB):
                    pm = psum_mm.tile([P, FB], dt.float32, name="mm", tag="mm")
                    for k in range(NKB):
                        nc.tensor.matmul(
                            pm[:],
                            w_in_sb[k][:, mb * P:(mb + 1) * P],
                            hT[k][:, fb * FB:(fb + 1) * FB],
                            start=(k == 0),
                            stop=(k == NKB - 1),
                        )
                    if mb < NPT:  # x half -> padded buffer (bf16)
                        nc.scalar.copy(
                            xpad[mb][:, PAD + fb * FB: PAD + (fb + 1) * FB], pm[:])
                    else:         # z half -> silu directly out of psum
                        pb = mb - NPT
                        nc.scalar.activation(
                            gz["f", pb][:, fb * FB:(fb + 1) * FB], pm[:], AF.Silu)
                        grev = gz["r", pb][:, ::-1]
                        nc.scalar.activation(
                            grev[:, fb * FB:(fb + 1) * FB], pm[:], AF.Silu)

            # -------------------------------- per-direction processing helpers
            xc = {}
            dt_sb = {}
            dtx = {}
            dxc = {}
            y = {}

            def conv_block(d):
                """causal depthwise conv + silu."""
                for pb in range(NPT):
                    cw = conv_w_sb[d, pb]
                    cb = conv_b_sb[d, pb]
                    if d == "f":
                        taps = [xpad[pb][:, k:k + L] for k in range(KCONV)]
                    else:
                        # reversed time: tap k reads xpad[2*PAD-k :][:L] reversed
                        taps = [xpad[pb][:, 2 * PAD - k: 2 * PAD - k + L][:, ::-1]
                                for k in range(KCONV)]
                    acc = cacc_pool.tile([P, L], dt.bfloat16, name="cacc", tag="cacc")
                    nc.scalar.activation(acc[:], taps[0], AF.Identity,
                                         bias=cb[:, 0:1], scale=cw[:, 0:1])
                    for k in range(1, KCONV):
                        acc2 = cacc_pool.tile([P, L], dt.bfloat16, name="cacc",
                                              tag="cacc")
                        nc.vector.scalar_tensor_tensor(
                            acc2[:], taps[k], cw[:, k:k + 1], acc[:],
                            OP.mult, OP.add)
                        acc = acc2
                    t = xc_pool.tile([P, L], dt.bfloat16, name="xc", tag="xc")
                    nc.scalar.activation(t[:], acc[:], AF.Silu)
                    xc[d, pb] = t

            def xproj_block(d, di):
                for fb in range(NFB):
                    pm = psum_mm.tile([NXP, FB], dt.float32, name="mmx", tag="mm")
                    for pb in range(NPT):
                        nc.tensor.matmul(
                            pm[:],
                            w_x_sb[d, pb][:],
                            xc[d, pb][:, fb * FB:(fb + 1) * FB],
                            start=(pb == 0),
                            stop=(pb == NPT - 1),
                        )
                    xev = xev_pool.tile([NXP, FB], dt.float32, name="xev", tag="xev")
                    nc.scalar.copy(xev[:], pm[:])
                    nc.sync.dma_start(
                        xdbl_part[di * NXP:(di + 1) * NXP, fb * FB:(fb + 1) * FB],
                        xev[:])

            xdbl = {}
            bc16 = {}

            def dt_block(d):
                """Load x_dbl, cast B/C to bf16, dt_proj + softplus, dtx, dxc."""
                xdbl[d] = xdbl_pool.tile([NXP, L], dt.float32, name="xdbl",
                                         tag="xdbl")
                di = 0 if d == "f" else 1
                nc.sync.dma_start(xdbl[d][:],
                                  xdbl_sum[di * NXP:(di + 1) * NXP, :])
                bc16[d] = bc16_pool.tile([2 * NST, L], dt.bfloat16, name="bc16",
                                         tag="bc16")
                nc.scalar.copy(bc16[d][:], xdbl[d][RK:NXP, :])
                nc.sync.dma_start(dram_bc16[d][:], bc16[d][:])
                for pb in range(NPT):
                    t = dt_pool.tile([P, L], dt.float32, name="dtt", tag="dtt")
                    for fb in range(NFB):
                        pm = psum_mm.tile([P, FB], dt.float32, name="mm", tag="mm")
                        nc.tensor.matmul(
                            pm[:],
                            w_dt_sb[d][:, pb * P:(pb + 1) * P],
                            xdbl[d][0:RK, fb * FB:(fb + 1) * FB],
                            start=True, stop=True)
                        et = etmp_pool.tile([P, FB], dt.float32, name="etmp",
                                            tag="etmp")
                        nc.scalar.activation(
                            et[:], pm[:], AF.Exp, bias=dt_b_sb[d, pb][:, 0:1])
                        nc.scalar.activation(
                            t[:, fb * FB:(fb + 1) * FB], et[:], AF.Ln, bias=1.0)
                    dt_sb[d, pb] = t
                    tx = dtx_pool.tile([P, L], dt.bfloat16, name="dtx", tag="dtx")
                    nc.vector.tensor_mul(tx[:], dt_sb[d, pb][:], xc[d, pb][:])
                    dtx[d, pb] = tx
                    dc = dxc_pool.tile([P, L], dt.bfloat16, name="dxc", tag="dxc")
                    nc.vector.tensor_scalar_mul(
                        dc[:], xc[d, pb][:], D_sb[d, pb][:, 0:1])
                    dxc[d, pb] = dc

            def scan_block(d):
                """Selective scan for one direction; writes y[d, pb]."""
                if skip_scan:
                    for pb in range(NPT):
                        yt = y_pool.tile([P, L], dt.bfloat16, name="y", tag="y")
                        nc.vector.tensor_mul(yt[:], dtx[d, pb][:], gz[d, pb][:])
                        y[d, pb] = yt
                    return
                stacks = {(pb, c): [] for pb in range(NPT) for c in range(NHC)}

                def push(pb, c, tile_, lvl):
                    st = stacks[pb, c]
                    while st and st[-1][1] == lvl:
                        other, _ = st.pop()
                        s = hc_pool.tile([P, HCCH], dt.bfloat16, name="hc", tag="hc")
                        nc.vector.tensor_add(s[:], other[:], tile_[:])
                        tile_ = s
                        lvl += 1
                    st.append((tile_, lvl))

                for n in range(NST):
                    rb = dram_bc16[d][n:n + 1, :]
                    rc = dram_bc16[d][NST + n:NST + n + 1, :]
                    bb = bbc_pool.tile([P, L], dt.bfloat16, name="bbc", tag="bbc")
                    nc.sync.dma_start(
                        bb[:], bass.AP(rb.tensor, rb.offset, [[0, P], [1, L]]))
                    cbt = cbc_pool.tile([P, L], dt.bfloat16, name="cbc", tag="cbc")
                    nc.sync.dma_start(
                        cbt[:], bass.AP(rc.tensor, rc.offset, [[0, P], [1, L]]))
                    for pb in range(NPT):
                        da = da_pool.tile([P, L], dt.float32, name="da", tag="da")
                        nc.scalar.activation(
                            da[:], dt_sb[d, pb][:], AF.Exp,
                            scale=A_sb[d, pb][:, n:n + 1])
                        dbx = dbx_pool.tile([P, L], dt.bfloat16, name="dbx",
                                            tag="dbx")
                        nc.gpsimd.tensor_tensor(dbx[:], dtx[d, pb][:], bb[:],
                                                OP.mult)
                        h = h_pool.tile([P, L], dt.bfloat16, name="h", tag="h")
                        nc.vector.tensor_tensor_scan(
                            h[:], da[:], dbx[:], 0.0, OP.mult, OP.add)
                        for c in range(NHC):
                            sl = slice(c * HCCH, (c + 1) * HCCH)
                            hc = hc_pool.tile([P, HCCH], dt.bfloat16, name="hc",
                                              tag="hc")
                            nc.vector.tensor_mul(hc[:], h[:, sl], cbt[:, sl])
                            push(pb, c, hc, 0)
                for pb in range(NPT):
                    yt = y_pool.tile([P, L], dt.bfloat16, name="y", tag="y")
                    gzt = gz[d, pb]
                    for c in range(NHC):
                        sl = slice(c * HCCH, (c + 1) * HCCH)
                        # collapse the counter stack (+ the D*x leaf)
                        st = stacks[pb, c]
                        acc_ap = dxc[d, pb][:, sl]
                        while st:
                            t_, _ = st.pop()
                            s = hc_pool.tile([P, HCCH], dt.bfloat16, name="hc",
                                             tag="hc")
                            nc.vector.tensor_add(s[:], t_[:], acc_ap)
                            acc_ap = s[:]
                        nc.vector.tensor_mul(yt[:, sl], acc_ap, gzt[:, sl])
                    y[d, pb] = yt

            ctxA.close()
            for di, d in enumerate(("f", "r")):
                conv_block(d)
                xproj_block(d, di)
            ctxB.close()

            nc.gpsimd.collective_compute(
                "AllReduce",
                OP.add,
                replica_groups=[list(range(num_cores))],
                ins=[xdbl_part[:].opt()],
                outs=[xdbl_sum[:].opt()],
            )

            # scan-phase pools (allocated after the stage-1/2 pools freed)
            etmp_pool = ctx.enter_context(tc.tile_pool(name="etmpp", bufs=2))
            dt_pool = ctx.enter_context(tc.tile_pool(name="dtp", bufs=2))
            dtx_pool = ctx.enter_context(tc.tile_pool(name="dtxp", bufs=2))
            dxc_pool = ctx.enter_context(tc.tile_pool(name="dxcp", bufs=2))
            xdbl_pool = ctx.enter_context(tc.tile_pool(name="xdblp", bufs=1))
            bc16_pool = ctx.enter_context(tc.tile_pool(name="bc16p", bufs=1))
            bbc_pool = ctx.enter_context(tc.tile_pool(name="bbcp", bufs=2))
            cbc_pool = ctx.enter_context(tc.tile_pool(name="cbcp", bufs=2))
            da_pool = ctx.enter_context(tc.tile_pool(name="dap", bufs=2))
            dbx_pool = ctx.enter_context(tc.tile_pool(name="dbxp", bufs=2))
            h_pool = ctx.enter_context(tc.tile_pool(name="hp", bufs=2))
            hc_pool = ctx.enter_context(tc.tile_pool(name="hcp", bufs=20))
            y_pool = ctx.enter_context(tc.tile_pool(name="yp", bufs=4))

            for d in ("f", "r"):
                dt_block(d)
                scan_block(d)

            # ------------------------------------------- out_proj + RS
            for tb in range(L // P):
                ycb = {}
                for pb in range(NPT):
                    yc = yc_pool.tile([P, P], dt.bfloat16, name="ycb", tag="ycb")
                    nc.vector.tensor_add(
                        yc[:], y["f", pb][:, tb * P:(tb + 1) * P],
                        y["r", pb][:, ::-1][:, tb * P:(tb + 1) * P])
                    ycb[pb] = yc
                for fb in range(DM // FB):
                    pm = psum_mm.tile([P, FB], dt.float32, name="mm", tag="mm")
                    for pb in range(NPT):
                        nc.tensor.matmul(
                            pm[:],
                            ycb[pb][:],
                            w_out_sb[pb][:, fb * FB:(fb + 1) * FB],
                            start=(pb == 0),
                            stop=(pb == NPT - 1),
                        )
                    oev = oev_pool.tile([P, FB], dt.float32, name="oev", tag="oev")
                    nc.scalar.copy(oev[:], pm[:])
                    nc.sync.dma_start(
                        pout[tb * P:(tb + 1) * P, fb * FB:(fb + 1) * FB], oev[:])

            nc.gpsimd.collective_compute(
                "ReduceScatter",
                OP.add,
                replica_groups=[list(range(num_cores))],
                ins=[pout[:].opt()],
                outs=[pout_rs[:].opt()],
            )
            nc.sync.dma_start(out[:], pout_rs[:])

    return nc


# ---------------------------------------------------------------- host side
def _make_in_maps(inputs):
    """Slice/transpose the full inputs into per-core input dicts."""
    h = np.ascontiguousarray(np.asarray(inputs["hidden_states"],
                                        dtype=np.float32).reshape(L, DM))
    w_in = np.asarray(inputs["in_proj_w"], dtype=np.float32)     # (2DI, DM)
    w_out = np.asarray(inputs["out_proj_w"], dtype=np.float32)   # (DM, DI)
    ident = np.eye(P, dtype=np.float32)

    in_maps = []
    for c in range(NCORES):
        sl = slice(c * CH, (c + 1) * CH)
        m = {"hidden": h, "ident": ident}
        w_slice = np.concatenate(
            [w_in[sl, :], w_in[DI + c * CH: DI + (c + 1) * CH, :]], axis=0)
        m["w_inT"] = np.ascontiguousarray(
            w_slice.T).astype(ml_dtypes.bfloat16)                 # (DM, 2CH)
        m["w_outT"] = np.ascontiguousarray(
            w_out[:, sl].T).astype(ml_dtypes.bfloat16)            # (CH, DM)
        for d, tag in (("f", "_f"), ("r", "_r")):
            w_x = np.asarray(inputs[f"x_proj_w{tag}"], dtype=np.float32)
            m[f"w_xT_{d}"] = np.ascontiguousarray(
                w_x[:, sl].T).astype(ml_dtypes.bfloat16)          # (CH, 96)
            w_dt = np.asarray(inputs[f"dt_proj_w{tag}"], dtype=np.float32)
            m[f"w_dtT_{d}"] = np.ascontiguousarray(w_dt[sl, :].T)  # (RK, CH)
            m[f"conv_w_{d}"] = np.ascontiguousarray(
                np.asarray(inputs[f"conv_w{tag}"], dtype=np.float32)[sl, :])
            m[f"conv_b_{d}"] = np.ascontiguousarray(
                np.asarray(inputs[f"conv_b{tag}"], dtype=np.float32)[sl, None])
            m[f"dt_b_{d}"] = np.ascontiguousarray(
                np.asarray(inputs[f"dt_proj_b{tag}"], dtype=np.float32)[sl, None])
            m[f"A_{d}"] = np.ascontiguousarray(
                -np.exp(np.asarray(inputs[f"A_log{tag}"], dtype=np.float32)[sl, :]))
            m[f"D_{d}"] = np.ascontiguousarray(
                np.asarray(inputs[f"D{tag}"], dtype=np.float32)[sl, None])
        in_maps.append(m)
    return in_maps


_CACHED = {}


def _install_ntff_hook_shim():
    """The agent image's antenv lacks axon_hooks; provide it and register
    the ctypes-based NTFF profile hook from trn_agent_boot."""
    import types
    try:
        import antenv.axon_hooks  # noqa: F401
        return
    except ImportError:
        pass
    import antenv
    mod = types.ModuleType("antenv.axon_hooks")
    _state = {"h": None}
    mod.get_axon_ntff_profile_hook = lambda: _state["h"]
    mod.set_axon_ntff_profile_hook = lambda h: _state.__setitem__("h", h)
    sys.modules["antenv.axon_hooks"] = mod
    antenv.axon_hooks = mod
    try:
        from trn_agent_boot.trn_boot import _ntff_profile_via_ctypes
        hook = _ntff_profile_via_ctypes("/opt/axon/libaxon_pjrt.so")
        if hook is not None:
            mod.set_axon_ntff_profile_hook(hook)
    except Exception:
        pass


def _install_hook_err_capture():
    """Wrap the neuronx_cc hook so compile errors land in hook_err.log
    instead of being swallowed by the PJRT boundary."""
    import traceback
    import concourse.bass2jax as b2j
    if getattr(b2j, "_err_capture_installed", False):
        return
    orig = b2j.neuronx_cc_hook

    def wrapped(*a):
        try:
            return orig(*a)
        except Exception:
            with open("/tmp/hook_err.log", "w") as f:
                f.write(traceback.format_exc())
            raise

    b2j.neuronx_cc_hook = wrapped
    b2j._err_capture_installed = True


def kernel(**inputs):
    from concourse.bass_utils import run_bass_kernel_spmd

    _install_ntff_hook_shim()
    _install_hook_err_capture()

    if "nc" not in _CACHED:
        from concourse.bass_interp import get_hw_module
        nc = build_program(
            skip_scan=bool(int(os.environ.get("KERNEL_SKIP_SCAN", "0"))))
        nc.finalize()  # bacc: register allocation, library/ACT-table loads
        nc.m = get_hw_module(nc.m)  # strip sim-only callback instructions
        _CACHED["nc"] = nc
    nc = _CACHED["nc"]

    in_maps = _make_in_maps(inputs)
    res = run_bass_kernel_spmd(
        nc, in_maps, core_ids=list(range(NCORES)),
        trace=bool(int(os.environ.get("KERNEL_TRACE", "0"))),
    )
    _CACHED["last_result"] = res
    outs = [res.results[c]["out"] for c in range(NCORES)]
    full = np.concatenate(outs, axis=0).reshape(1, L, DM).astype(np.float32)
    return full


if __name__ == "__main__":
    nc = build_program()
    try:
        n = sum(len(bb.instructions) for bb in nc.main_func.blocks)
    except Exception:
        n = "?"
    print("build ok; instructions:", n)



# revision 10
# speedup vs baseline: 1.1378x; 1.1378x over previous
"""BiMamba Trainium2 kernel (v2).

Sharding: each of the 8 cores owns a 256-channel slice of d_inner for BOTH
directions (fwd+rev share in_proj/out_proj, so the reversed direction's
in_proj output is just a flipped view of the forward one).

v2 changes vs v1:
  - x-half of in_proj runs first so conv/x_proj/AllReduce start earlier;
    z-half + silu gating overlap the AllReduce.
  - AllReduce payload in bf16 (halves collective time).
  - dt/x_dbl pipeline in bf16; ACT exp/ln calls batched per function to
    avoid ACT table thrash.
  - Scan-phase reduction over states uses two accumulator chains per
    (dir, ptile): one on DVE, one on GpSimd, merged at the end. Keeps
    both engines ~equally loaded (DVE also owns the 64 scans + hC mults,
    GpSimd owns the dbx mults).
  - full-L hC tiles (no 1024-chunking).
  - out_proj ReduceScatter split in 2 overlapped chunks (host reassembles
    the permuted row blocks).
"""

import os
import sys

sys.path.insert(0, "/opt/trn_rl_repo")

import numpy as np
import ml_dtypes

# ---------------------------------------------------------------- constants
P = 128           # partitions
L = 2048          # sequence length
DM = 1024         # d_model
DI = 2048         # d_inner
NST = 16          # d_state
RK = 64           # dt_rank
KCONV = 4         # conv width
NCORES = 8
CH = DI // NCORES          # channels per core per direction = 256
NPT = CH // P              # channel ptiles per core = 2
FB = 512                   # matmul moving free chunk
NFB = L // FB              # 4
PAD = KCONV - 1            # causal pad = 3
NXP = RK + 2 * NST         # 96
NRS = 2                    # ReduceScatter chunks (token blocks)
RSROWS = L // NRS          # rows per RS chunk = 1024


def build_program(num_cores=NCORES, skip_scan=False):
    """Build the SPMD Bass program (same NEFF on every core)."""
    import concourse.bass as bass
    import concourse.mybir as mybir
    import concourse.tile as tile
    from concourse import bacc
    from contextlib import ExitStack

    dt = mybir.dt
    AF = mybir.ActivationFunctionType
    OP = mybir.AluOpType

    nc = bacc.Bacc(
        "TRN2",
        target_bir_lowering=False,
        debug=False,
        enable_asserts=False,
        num_devices=num_cores,
    )

    # ------------------------------------------------------------- dram I/O
    hidden = nc.dram_tensor("hidden", [L, DM], dt.float32, kind="ExternalInput")
    w_inT = nc.dram_tensor("w_inT", [DM, 2 * CH], dt.bfloat16, kind="ExternalInput")
    w_outT = nc.dram_tensor("w_outT", [CH, DM], dt.bfloat16, kind="ExternalInput")
    w_xT = {}
    w_dtT = {}
    conv_w = {}
    conv_b = {}
    dt_b = {}
    A_in = {}
    D_in = {}
    for d in ("f", "r"):
        w_xT[d] = nc.dram_tensor(f"w_xT_{d}", [CH, NXP], dt.bfloat16,
                                 kind="ExternalInput")
        w_dtT[d] = nc.dram_tensor(f"w_dtT_{d}", [RK, CH], dt.bfloat16,
                                  kind="ExternalInput")
        conv_w[d] = nc.dram_tensor(f"conv_w_{d}", [CH, KCONV], dt.float32,
                                   kind="ExternalInput")
        conv_b[d] = nc.dram_tensor(f"conv_b_{d}", [CH, 1], dt.float32,
                                   kind="ExternalInput")
        dt_b[d] = nc.dram_tensor(f"dt_b_{d}", [CH, 1], dt.float32,
                                 kind="ExternalInput")
        A_in[d] = nc.dram_tensor(f"A_{d}", [CH, NST], dt.float32,
                                 kind="ExternalInput")
        D_in[d] = nc.dram_tensor(f"D_{d}", [CH, 1], dt.float32,
                                 kind="ExternalInput")
    ident = nc.dram_tensor("ident", [P, P], dt.float32, kind="ExternalInput")
    out = nc.dram_tensor("out", [L // num_cores, DM], dt.float32,
                         kind="ExternalOutput")

    NKB = DM // P  # 8
    NTT = L // P   # 16
    WPAD = L + 2 * PAD  # padded x width (zeros both ends for the rev conv)

    with tile.TileContext(nc) as tc:
        ctx = ExitStack()
        with ctx:
            dram = ctx.enter_context(tc.tile_pool(name="dram", bufs=1, space="DRAM"))
            consts = ctx.enter_context(tc.tile_pool(name="consts", bufs=1))
            psum_mm = ctx.enter_context(
                tc.tile_pool(name="psum_mm", bufs=3, space="PSUM"))

            # ---------------------------------------------------- constants
            ident_sb = consts.tile([P, P], dt.float32)
            nc.sync.dma_start(ident_sb[:], ident[:])
            conv_w_sb = {}
            conv_b_sb = {}
            dt_b_sb = {}
            A_sb = {}
            D_sb = {}
            for d in ("f", "r"):
                for pb in range(NPT):
                    ps = slice(pb * P, (pb + 1) * P)
                    for nm, store, src, shape in (
                        ("cw", conv_w_sb, conv_w, [P, KCONV]),
                        ("cb", conv_b_sb, conv_b, [P, 1]),
                        ("db", dt_b_sb, dt_b, [P, 1]),
                        ("A", A_sb, A_in, [P, NST]),
                        ("Dc", D_sb, D_in, [P, 1]),
                    ):
                        t = consts.tile(shape, dt.float32, name=f"{nm}{d}{pb}",
                                        tag=f"{nm}{d}{pb}")
                        nc.sync.dma_start(t[:], src[d][ps, :])
                        store[d, pb] = t
            w_dt_sb = {}
            for d in ("f", "r"):
                w_dt_sb[d] = consts.tile([RK, CH], dt.bfloat16, name=f"wdt{d}",
                                         tag=f"wdt{d}")
                nc.sync.dma_start(w_dt_sb[d][:], w_dtT[d][:])
            w_x_sb = {}
            for d in ("f", "r"):
                for pb in range(NPT):
                    t = consts.tile([P, NXP], dt.bfloat16, name=f"wx{d}{pb}",
                                    tag=f"wx{d}{pb}")
                    nc.sync.dma_start(t[:], w_xT[d][pb * P:(pb + 1) * P, :])
                    w_x_sb[d, pb] = t
            w_out_sb = []
            for pb in range(NPT):
                t = consts.tile([P, DM], dt.bfloat16, name=f"wo{pb}", tag=f"wo{pb}")
                nc.sync.dma_start(t[:], w_outT[pb * P:(pb + 1) * P, :])
                w_out_sb.append(t)

            # persistent activation buffers
            gz_pool = ctx.enter_context(tc.tile_pool(name="gzp", bufs=1))
            gz = {}
            for d in ("f", "r"):
                for pb in range(NPT):
                    gz[d, pb] = gz_pool.tile([P, L], dt.bfloat16,
                                             name=f"gz{d}{pb}", tag=f"gz{d}{pb}")
            yc_pool = ctx.enter_context(tc.tile_pool(name="ycp", bufs=4))
            oev_pool = ctx.enter_context(tc.tile_pool(name="oevp", bufs=3))
            dt_pool = ctx.enter_context(tc.tile_pool(name="dtp", bufs=1))
            dtx_pool = ctx.enter_context(tc.tile_pool(name="dtxp", bufs=4))
            dxc_pool = ctx.enter_context(tc.tile_pool(name="dxcp", bufs=4))

            xdbl_part = dram.tile([2 * NXP, L], dt.bfloat16)
            xdbl_sum = dram.tile([2 * NXP, L], dt.bfloat16, addr_space="Shared")
            pout = dram.tile([L, DM], dt.float32)
            pout_rs = dram.tile([L // num_cores, DM], dt.float32)

            # stage-limited pools (freed once early phases are emitted)
            ctxB = ExitStack()
            xpad_pool = ctxB.enter_context(tc.tile_pool(name="xpadp", bufs=1))
            xc_pool = ctxB.enter_context(tc.tile_pool(name="xcp", bufs=4))
            cacc_pool = ctxB.enter_context(tc.tile_pool(name="caccp", bufs=3))
            xev_pool = ctxB.enter_context(tc.tile_pool(name="xevp", bufs=2))
            et_pool = ctxB.enter_context(tc.tile_pool(name="etp", bufs=8))
            xdbl_pool = ctxB.enter_context(tc.tile_pool(name="xdblp", bufs=1))
            ctxA = ExitStack()
            hT_pool = ctxA.enter_context(tc.tile_pool(name="hTp", bufs=1))
            hnat_pool = ctxA.enter_context(tc.tile_pool(name="hnatp", bufs=4))
            w_in_pool = ctxA.enter_context(tc.tile_pool(name="winp", bufs=1))

            # ------------------------------------------- stage 1: hT = hidden^T
            psum_tp = ctxA.enter_context(
                tc.tile_pool(name="psum_tp", bufs=3, space="PSUM"))
            hT = [hT_pool.tile([P, L], dt.bfloat16, name=f"hT{k}", tag=f"hT{k}")
                  for k in range(NKB)]
            for q in range(NTT // 4):
                hn = []
                for j in range(4):
                    t = hnat_pool.tile([P, DM], dt.float32, name="hnat", tag="hnat")
                    nc.sync.dma_start(
                        t[:], hidden[(q * 4 + j) * P:(q * 4 + j + 1) * P, :])
                    hn.append(t)
                for kb in range(NKB):
                    pt = psum_tp.tile([P, 4 * P], dt.float32, name="tp", tag="tp")
                    for j in range(4):
                        nc.tensor.transpose(
                            pt[:, j * P:(j + 1) * P],
                            hn[j][:, kb * P:(kb + 1) * P],
                            ident_sb[:],
                        )
                    nc.scalar.copy(hT[kb][:, q * 4 * P:(q + 1) * 4 * P], pt[:])

            # ------------------------------------------- stage 2: in_proj
            w_in_sb = [w_in_pool.tile([P, 2 * CH], dt.bfloat16, name=f"win{k}",
                                      tag=f"win{k}") for k in range(NKB)]
            for k in range(NKB):
                nc.sync.dma_start(w_in_sb[k][:], w_inT[k * P:(k + 1) * P, :])

            xpad = [xpad_pool.tile([P, WPAD], dt.bfloat16, name=f"xpad{pb}",
                                   tag=f"xpad{pb}") for pb in range(NPT)]
            for pb in range(NPT):
                nc.vector.memset(xpad[pb][:, 0:PAD], 0.0)
                nc.vector.memset(xpad[pb][:, PAD + L:WPAD], 0.0)

            # x half first (feeds conv -> x_proj -> AllReduce critical path)
            for mb in range(NPT):
                for fb in range(NFB):
                    pm = psum_mm.tile([P, FB], dt.float32, name="mm", tag="mm")
                    for k in range(NKB):
                        nc.tensor.matmul(
                            pm[:],
                            w_in_sb[k][:, mb * P:(mb + 1) * P],
                            hT[k][:, fb * FB:(fb + 1) * FB],
                            start=(k == 0),
                            stop=(k == NKB - 1),
                        )
                    nc.scalar.copy(
                        xpad[mb][:, PAD + fb * FB: PAD + (fb + 1) * FB], pm[:])

            xc = {}

            def conv_block(d):
                """causal depthwise conv + silu."""
                for pb in range(NPT):
                    cw = conv_w_sb[d, pb]
                    cb = conv_b_sb[d, pb]
                    if d == "f":
                        taps = [xpad[pb][:, k:k + L] for k in range(KCONV)]
                    else:
                        taps = [xpad[pb][:, 2 * PAD - k: 2 * PAD - k + L][:, ::-1]
                                for k in range(KCONV)]
                    acc = cacc_pool.tile([P, L], dt.bfloat16, name="cacc", tag="cacc")
                    nc.scalar.activation(acc[:], taps[0], AF.Identity,
                                         bias=cb[:, 0:1], scale=cw[:, 0:1])
                    for k in range(1, KCONV):
                        acc2 = cacc_pool.tile([P, L], dt.bfloat16, name="cacc",
                                              tag="cacc")
                        nc.vector.scalar_tensor_tensor(
                            acc2[:], taps[k], cw[:, k:k + 1], acc[:],
                            OP.mult, OP.add)
                        acc = acc2
                    t = xc_pool.tile([P, L], dt.bfloat16, name="xc", tag="xc")
                    nc.scalar.activation(t[:], acc[:], AF.Silu)
                    xc[d, pb] = t

            def xproj_block(d, di):
                for fb in range(NFB):
                    pm = psum_mm.tile([NXP, FB], dt.float32, name="mmx", tag="mm")
                    for pb in range(NPT):
                        nc.tensor.matmul(
                            pm[:],
                            w_x_sb[d, pb][:],
                            xc[d, pb][:, fb * FB:(fb + 1) * FB],
                            start=(pb == 0),
                            stop=(pb == NPT - 1),
                        )
                    xev = xev_pool.tile([NXP, FB], dt.bfloat16, name="xev",
                                        tag="xev")
                    nc.scalar.copy(xev[:], pm[:])
                    nc.sync.dma_start(
                        xdbl_part[di * NXP:(di + 1) * NXP, fb * FB:(fb + 1) * FB],
                        xev[:])

            for di, d in enumerate(("f", "r")):
                conv_block(d)
                xproj_block(d, di)

            # AllReduce ASAP (bf16 payload)
            nc.gpsimd.collective_compute(
                "AllReduce",
                OP.add,
                replica_groups=[list(range(num_cores))],
                ins=[xdbl_part[:].opt()],
                outs=[xdbl_sum[:].opt()],
            )

            # ------------------- z half of in_proj + silu gating (overlaps AR)
            for mb in range(NPT, 2 * NPT):
                for fb in range(NFB):
                    pm = psum_mm.tile([P, FB], dt.float32, name="mm", tag="mm")
                    for k in range(NKB):
                        nc.tensor.matmul(
                            pm[:],
                            w_in_sb[k][:, mb * P:(mb + 1) * P],
                            hT[k][:, fb * FB:(fb + 1) * FB],
                            start=(k == 0),
                            stop=(k == NKB - 1),
                        )
                    pb = mb - NPT
                    nc.scalar.activation(
                        gz["f", pb][:, fb * FB:(fb + 1) * FB], pm[:], AF.Silu)
                    grev = gz["r", pb][:, ::-1]
                    nc.scalar.activation(
                        grev[:, fb * FB:(fb + 1) * FB], pm[:], AF.Silu)

            ctxA.close()

            dt_sb = {}
            dtx = {}
            dxc = {}
            xdbl = {}
            y = {}

            def dt_block(d):
                """Load x_dbl dt-rows, dt_proj + softplus, dtx, dxc."""
                xdbl[d] = xdbl_pool.tile([RK, L], dt.bfloat16, name="xdbl",
                                         tag=f"xdbl{d}")
                di = 0 if d == "f" else 1
                nc.sync.dma_start(xdbl[d][:],
                                  xdbl_sum[di * NXP:di * NXP + RK, :])
                # dt_proj matmuls + batched Exp then batched Ln (avoid ACT
                # table thrash)
                ets = {}
                for pb in range(NPT):
                    for fb in range(NFB):
                        pm = psum_mm.tile([P, FB], dt.float32, name="mm",
                                          tag="mm")
                        nc.tensor.matmul(
                            pm[:],
                            w_dt_sb[d][:, pb * P:(pb + 1) * P],
                            xdbl[d][0:RK, fb * FB:(fb + 1) * FB],
                            start=True, stop=True)
                        et = et_pool.tile([P, FB], dt.float32, name="etmp",
                                          tag="etmp")
                        nc.scalar.activation(
                            et[:], pm[:], AF.Exp, bias=dt_b_sb[d, pb][:, 0:1])
                        ets[pb, fb] = et
                for pb in range(NPT):
                    t = dt_pool.tile([P, L], dt.bfloat16, name="dtt",
                                     tag=f"dtt{d}{pb}")
                    for fb in range(NFB):
                        nc.scalar.activation(
                            t[:, fb * FB:(fb + 1) * FB], ets[pb, fb][:],
                            AF.Ln, bias=1.0)
                    dt_sb[d, pb] = t
                    tx = dtx_pool.tile([P, L], dt.bfloat16, name="dtx", tag="dtx")
                    nc.vector.tensor_mul(tx[:], dt_sb[d, pb][:], xc[d, pb][:])
                    dtx[d, pb] = tx
                    dc = dxc_pool.tile([P, L], dt.bfloat16, name="dxc", tag="dxc")
                    nc.vector.tensor_scalar_mul(
                        dc[:], xc[d, pb][:], D_sb[d, pb][:, 0:1])
                    dxc[d, pb] = dc

            def scan_block(d):
                """Selective scan for one direction; writes y[d, pb].

                Reduction over the 16 states runs as two accumulator chains
                per ptile: DVE chain (seeded with dxc) and GpSimd chain.
                GPS_N states go to the gpsimd chain.
                """
                if skip_scan:
                    for pb in range(NPT):
                        yt = y_pool.tile([P, L], dt.bfloat16, name="y", tag="y")
                        nc.vector.tensor_mul(yt[:], dtx[d, pb][:], gz[d, pb][:])
                        y[d, pb] = yt
                    return
                GPS_CHAIN = (3, 5, 7, 11, 13, 15)  # states whose hC-add runs on gpsimd
                di = 0 if d == "f" else 1
                accv = {}
                accg = {}
                for n in range(NST):
                    rb = xdbl_sum[di * NXP + RK + n: di * NXP + RK + n + 1, :]
                    rc = xdbl_sum[di * NXP + RK + NST + n:
                                  di * NXP + RK + NST + n + 1, :]
                    bb = bbc_pool.tile([P, L], dt.bfloat16, name="bbc", tag="bbc")
                    nc.sync.dma_start(
                        bb[:], bass.AP(rb.tensor, rb.offset, [[0, P], [1, L]]))
                    cbt = cbc_pool.tile([P, L], dt.bfloat16, name="cbc", tag="cbc")
                    nc.sync.dma_start(
                        cbt[:], bass.AP(rc.tensor, rc.offset, [[0, P], [1, L]]))
                    for pb in range(NPT):
                        da = da_pool.tile([P, L], dt.float32, name="da", tag="da")
                        nc.scalar.activation(
                            da[:], dt_sb[d, pb][:], AF.Exp,
                            scale=A_sb[d, pb][:, n:n + 1])
                        dbx = dbx_pool.tile([P, L], dt.bfloat16, name="dbx",
                                            tag="dbx")
                        nc.gpsimd.tensor_tensor(dbx[:], dtx[d, pb][:], bb[:],
                                                OP.mult)
                        h = h_pool.tile([P, L], dt.bfloat16, name="h", tag="h")
                        nc.vector.tensor_tensor_scan(
                            h[:], da[:], dbx[:], 0.0, OP.mult, OP.add)
                        # hC product; seeds are written straight into a chain
                        # buffer so the pool ping-pong never clobbers a live
                        # accumulator.
                        gps_n = n in GPS_CHAIN
                        if gps_n:
                            pool_, tag_, acc_ = accg_pool, f"accg{pb}", accg
                        else:
                            pool_, tag_, acc_ = accv_pool, f"accv{pb}", accv
                        if (d, pb) not in acc_:
                            hc = pool_.tile([P, L], dt.bfloat16, name="hc",
                                            tag=tag_)
                            nc.vector.tensor_mul(hc[:], h[:], cbt[:])
                            acc_[d, pb] = hc
                        else:
                            hc = hc_pool.tile([P, L], dt.bfloat16, name="hc",
                                              tag="hc")
                            nc.vector.tensor_mul(hc[:], h[:], cbt[:])
                            s = pool_.tile([P, L], dt.bfloat16, name="acc",
                                           tag=tag_)
                            if gps_n:
                                nc.gpsimd.tensor_tensor(
                                    s[:], acc_[d, pb][:], hc[:], OP.add)
                            else:
                                nc.vector.tensor_add(
                                    s[:], acc_[d, pb][:], hc[:])
                            acc_[d, pb] = s
                for pb in range(NPT):
                    # merge chains + D*x + gate
                    m = accv_pool.tile([P, L], dt.bfloat16, name="accm",
                                       tag=f"accv{pb}")
                    nc.vector.tensor_add(m[:], accv[d, pb][:], accg[d, pb][:])
                    m2 = accg_pool.tile([P, L], dt.bfloat16, name="accm2",
                                        tag=f"accg{pb}")
                    nc.vector.tensor_add(m2[:], m[:], dxc[d, pb][:])
                    yt = y_pool.tile([P, L], dt.bfloat16, name="y", tag="y")
                    nc.vector.tensor_mul(yt[:], m2[:], gz[d, pb][:])
                    y[d, pb] = yt

            dt_block("f")
            dt_block("r")
            ctxB.close()

            # ------------------------------------------- scan pools
            bbc_pool = ctx.enter_context(tc.tile_pool(name="bbcp", bufs=2))
            cbc_pool = ctx.enter_context(tc.tile_pool(name="cbcp", bufs=2))
            da_pool = ctx.enter_context(tc.tile_pool(name="dap", bufs=2))
            dbx_pool = ctx.enter_context(tc.tile_pool(name="dbxp", bufs=2))
            h_pool = ctx.enter_context(tc.tile_pool(name="hp", bufs=2))
            hc_pool = ctx.enter_context(tc.tile_pool(name="hcp", bufs=3))
            accv_pool = ctx.enter_context(tc.tile_pool(name="accvp", bufs=2))
            accg_pool = ctx.enter_context(tc.tile_pool(name="accgp", bufs=2))
            y_pool = ctx.enter_context(tc.tile_pool(name="yp", bufs=4))

            scan_block("f")
            scan_block("r")

            # ------------------------------------------- out_proj + RS
            for rs in range(NRS):
                for tbo in range(RSROWS // P):
                    tb = rs * (RSROWS // P) + tbo
                    ycb = {}
                    for pb in range(NPT):
                        yct = yc_pool.tile([P, P], dt.bfloat16, name="ycb",
                                           tag="ycb")
                        nc.vector.tensor_add(
                            yct[:], y["f", pb][:, tb * P:(tb + 1) * P],
                            y["r", pb][:, ::-1][:, tb * P:(tb + 1) * P])
                        ycb[pb] = yct
                    for fb in range(DM // FB):
                        pm = psum_mm.tile([P, FB], dt.float32, name="mm",
                                          tag="mm")
                        for pb in range(NPT):
                            nc.tensor.matmul(
                                pm[:],
                                ycb[pb][:],
                                w_out_sb[pb][:, fb * FB:(fb + 1) * FB],
                                start=(pb == 0),
                                stop=(pb == NPT - 1),
                            )
                        oev = oev_pool.tile([P, FB], dt.float32, name="oev",
                                            tag="oev")
                        nc.scalar.copy(oev[:], pm[:])
                        nc.sync.dma_start(
                            pout[tb * P:(tb + 1) * P, fb * FB:(fb + 1) * FB],
                            oev[:])
                # chunk RS as soon as its token rows are written
                nc.gpsimd.collective_compute(
                    "ReduceScatter",
                    OP.add,
                    replica_groups=[list(range(num_cores))],
                    ins=[pout[rs * RSROWS:(rs + 1) * RSROWS, :].opt()],
                    outs=[pout_rs[rs * (RSROWS // num_cores):
                                  (rs + 1) * (RSROWS // num_cores), :].opt()],
                )
                nc.sync.dma_start(
                    out[rs * (RSROWS // num_cores):
                        (rs + 1) * (RSROWS // num_cores), :],
                    pout_rs[rs * (RSROWS // num_cores):
                            (rs + 1) * (RSROWS // num_cores), :])

    return nc


# ---------------------------------------------------------------- host side
def _make_in_maps(inputs):
    """Slice/transpose the full inputs into per-core input dicts."""
    h = np.ascontiguousarray(np.asarray(inputs["hidden_states"],
                                        dtype=np.float32).reshape(L, DM))
    w_in = np.asarray(inputs["in_proj_w"], dtype=np.float32)     # (2DI, DM)
    w_out = np.asarray(inputs["out_proj_w"], dtype=np.float32)   # (DM, DI)
    ident = np.eye(P, dtype=np.float32)

    in_maps = []
    for c in range(NCORES):
        sl = slice(c * CH, (c + 1) * CH)
        m = {"hidden": h, "ident": ident}
        w_slice = np.concatenate(
            [w_in[sl, :], w_in[DI + c * CH: DI + (c + 1) * CH, :]], axis=0)
        m["w_inT"] = np.ascontiguousarray(
            w_slice.T).astype(ml_dtypes.bfloat16)                 # (DM, 2CH)
        m["w_outT"] = np.ascontiguousarray(
            w_out[:, sl].T).astype(ml_dtypes.bfloat16)            # (CH, DM)
        for d, tag in (("f", "_f"), ("r", "_r")):
            w_x = np.asarray(inputs[f"x_proj_w{tag}"], dtype=np.float32)
            m[f"w_xT_{d}"] = np.ascontiguousarray(
                w_x[:, sl].T).astype(ml_dtypes.bfloat16)          # (CH, 96)
            w_dt = np.asarray(inputs[f"dt_proj_w{tag}"], dtype=np.float32)
            m[f"w_dtT_{d}"] = np.ascontiguousarray(
                w_dt[sl, :].T).astype(ml_dtypes.bfloat16)         # (RK, CH)
            m[f"conv_w_{d}"] = np.ascontiguousarray(
                np.asarray(inputs[f"conv_w{tag}"], dtype=np.float32)[sl, :])
            m[f"conv_b_{d}"] = np.ascontiguousarray(
                np.asarray(inputs[f"conv_b{tag}"], dtype=np.float32)[sl, None])
            m[f"dt_b_{d}"] = np.ascontiguousarray(
                np.asarray(inputs[f"dt_proj_b{tag}"], dtype=np.float32)[sl, None])
            m[f"A_{d}"] = np.ascontiguousarray(
                -np.exp(np.asarray(inputs[f"A_log{tag}"], dtype=np.float32)[sl, :]))
            m[f"D_{d}"] = np.ascontiguousarray(
                np.asarray(inputs[f"D{tag}"], dtype=np.float32)[sl, None])
        in_maps.append(m)
    return in_maps


_CACHED = {}


def _install_ntff_hook_shim():
    """The agent image's antenv lacks axon_hooks; provide it and register
    the ctypes-based NTFF profile hook from trn_agent_boot."""
    import types
    try:
        import antenv.axon_hooks  # noqa: F401
        return
    except ImportError:
        pass
    import antenv
    mod = types.ModuleType("antenv.axon_hooks")
    _state = {"h": None}
    mod.get_axon_ntff_profile_hook = lambda: _state["h"]
    mod.set_axon_ntff_profile_hook = lambda h: _state.__setitem__("h", h)
    sys.modules["antenv.axon_hooks"] = mod
    antenv.axon_hooks = mod
    try:
        from trn_agent_boot.trn_boot import _ntff_profile_via_ctypes
        hook = _ntff_profile_via_ctypes("/opt/axon/libaxon_pjrt.so")
        if hook is not None:
            mod.set_axon_ntff_profile_hook(hook)
    except Exception:
        pass


def _install_hook_err_capture():
    """Wrap the neuronx_cc hook so compile errors land in hook_err.log
    instead of being swallowed by the PJRT boundary."""
    import traceback
    import concourse.bass2jax as b2j
    if getattr(b2j, "_err_capture_installed", False):
        return
    orig = b2j.neuronx_cc_hook

    def wrapped(*a):
        try:
            return orig(*a)
        except Exception:
            with open("/tmp/hook_err.log", "w") as f:
                f.write(traceback.format_exc())
            raise

    b2j.neuronx_cc_hook = wrapped
    b2j._err_capture_installed = True


def kernel(**inputs):
    from concourse.bass_utils import run_bass_kernel_spmd

    _install_ntff_hook_shim()
    _install_hook_err_capture()

    if "nc" not in _CACHED:
        from concourse.bass_interp import get_hw_module
        nc = build_program(
            skip_scan=bool(int(os.environ.get("KERNEL_SKIP_SCAN", "0"))))
        nc.finalize()  # bacc: register allocation, library/ACT-table loads
        nc.m = get_hw_module(nc.m)  # strip sim-only callback instructions
        _CACHED["nc"] = nc
    nc = _CACHED["nc"]

    in_maps = _make_in_maps(inputs)
    res = run_bass_kernel_spmd(
        nc, in_maps, core_ids=list(range(NCORES)),
        trace=bool(int(os.environ.get("KERNEL_TRACE", "0"))),
    )
    _CACHED["last_result"] = res
    # Chunked ReduceScatter permutes row ownership: core c's out rows are
    # [rs*RSROWS + c*(RSROWS/8) : +RSROWS/8) for each rs chunk.
    rows = RSROWS // NCORES
    full = np.empty((L, DM), dtype=np.float32)
    for c in range(NCORES):
        o = res.results[c]["out"]
        for rs in range(NRS):
            full[rs * RSROWS + c * rows: rs * RSROWS + (c + 1) * rows, :] = \
                o[rs * rows:(rs + 1) * rows, :]
    return full.reshape(1, L, DM)


if __name__ == "__main__":
    nc = build_program()
    try:
        n = sum(len(bb.instructions) for bb in nc.main_func.blocks)
    except Exception:
        n = "?"
    print("build ok; instructions:", n)


# revision 15
# speedup vs baseline: 1.2195x; 1.0718x over previous
"""BiMamba Trainium2 kernel (v2).

Sharding: each of the 8 cores owns a 256-channel slice of d_inner for BOTH
directions (fwd+rev share in_proj/out_proj, so the reversed direction's
in_proj output is just a flipped view of the forward one).

v2 changes vs v1:
  - x-half of in_proj runs first so conv/x_proj/AllReduce start earlier;
    z-half + silu gating overlap the AllReduce.
  - AllReduce payload in bf16 (halves collective time).
  - dt/x_dbl pipeline in bf16; ACT exp/ln calls batched per function to
    avoid ACT table thrash.
  - Scan-phase reduction over states uses two accumulator chains per
    (dir, ptile): one on DVE, one on GpSimd, merged at the end. Keeps
    both engines ~equally loaded (DVE also owns the 64 scans + hC mults,
    GpSimd owns the dbx mults).
  - full-L hC tiles (no 1024-chunking).
  - out_proj ReduceScatter split in 2 overlapped chunks (host reassembles
    the permuted row blocks).
"""

import os
import sys

sys.path.insert(0, "/opt/trn_rl_repo")

import numpy as np
import ml_dtypes

# ---------------------------------------------------------------- constants
P = 128           # partitions
L = 2048          # sequence length
DM = 1024         # d_model
DI = 2048         # d_inner
NST = 16          # d_state
RK = 64           # dt_rank
KCONV = 4         # conv width
NCORES = 8
CH = DI // NCORES          # channels per core per direction = 256
NPT = CH // P              # channel ptiles per core = 2
FB = 512                   # matmul moving free chunk
NFB = L // FB              # 4
PAD = KCONV - 1            # causal pad = 3
NXP = RK + 2 * NST         # 96
NRS = 4                    # ReduceScatter chunks (token blocks)
RSROWS = L // NRS          # rows per RS chunk = 1024


def build_program(num_cores=NCORES, skip_scan=False):
    """Build the SPMD Bass program (same NEFF on every core)."""
    import concourse.bass as bass
    import concourse.mybir as mybir
    import concourse.tile as tile
    from concourse import bacc
    from contextlib import ExitStack

    dt = mybir.dt
    AF = mybir.ActivationFunctionType
    OP = mybir.AluOpType

    nc = bacc.Bacc(
        "TRN2",
        target_bir_lowering=False,
        debug=False,
        enable_asserts=False,
        num_devices=num_cores,
    )

    # ------------------------------------------------------------- dram I/O
    hidden = nc.dram_tensor("hidden", [L, DM], dt.float32, kind="ExternalInput")
    w_inT = nc.dram_tensor("w_inT", [DM, 2 * CH], dt.bfloat16, kind="ExternalInput")
    w_outT = nc.dram_tensor("w_outT", [CH, DM], dt.bfloat16, kind="ExternalInput")
    w_xT = {}
    w_dtT = {}
    conv_w = {}
    conv_b = {}
    dt_b = {}
    A_in = {}
    D_in = {}
    for d in ("f", "r"):
        w_xT[d] = nc.dram_tensor(f"w_xT_{d}", [CH, NXP], dt.bfloat16,
                                 kind="ExternalInput")
        w_dtT[d] = nc.dram_tensor(f"w_dtT_{d}", [RK, CH], dt.bfloat16,
                                  kind="ExternalInput")
        conv_w[d] = nc.dram_tensor(f"conv_w_{d}", [CH, KCONV], dt.float32,
                                   kind="ExternalInput")
        conv_b[d] = nc.dram_tensor(f"conv_b_{d}", [CH, 1], dt.float32,
                                   kind="ExternalInput")
        dt_b[d] = nc.dram_tensor(f"dt_b_{d}", [CH, 1], dt.float32,
                                 kind="ExternalInput")
        A_in[d] = nc.dram_tensor(f"A_{d}", [CH, NST], dt.float32,
                                 kind="ExternalInput")
        D_in[d] = nc.dram_tensor(f"D_{d}", [CH, 1], dt.float32,
                                 kind="ExternalInput")
    ident = nc.dram_tensor("ident", [P, P], dt.float32, kind="ExternalInput")
    out = nc.dram_tensor("out", [L // num_cores, DM], dt.float32,
                         kind="ExternalOutput")

    NKB = DM // P  # 8
    NTT = L // P   # 16
    WPAD = L + 2 * PAD  # padded x width (zeros both ends for the rev conv)

    with tile.TileContext(nc) as tc:
        ctx = ExitStack()
        with ctx:
            dram = ctx.enter_context(tc.tile_pool(name="dram", bufs=1, space="DRAM"))
            consts = ctx.enter_context(tc.tile_pool(name="consts", bufs=1))
            psum_mm = ctx.enter_context(
                tc.tile_pool(name="psum_mm", bufs=3, space="PSUM"))

            # ---------------------------------------------------- constants
            ident_sb = consts.tile([P, P], dt.float32)
            nc.sync.dma_start(ident_sb[:], ident[:])
            # persistent activation buffers
            gz_pool = ctx.enter_context(tc.tile_pool(name="gzp", bufs=1))
            gz = {}
            for d in ("f", "r"):
                for pb in range(NPT):
                    gz[d, pb] = gz_pool.tile([P, L], dt.bfloat16,
                                             name=f"gz{d}{pb}", tag=f"gz{d}{pb}")
            yc_pool = ctx.enter_context(tc.tile_pool(name="ycp", bufs=4))
            oev_pool = ctx.enter_context(tc.tile_pool(name="oevp", bufs=3))
            dt_pool = ctx.enter_context(tc.tile_pool(name="dtp", bufs=1))
            dtx_pool = ctx.enter_context(tc.tile_pool(name="dtxp", bufs=4))
            dxc_pool = ctx.enter_context(tc.tile_pool(name="dxcp", bufs=4))

            xdbl_part = dram.tile([2 * NXP, L], dt.bfloat16)
            xdbl_sum = dram.tile([2 * NXP, L], dt.bfloat16, addr_space="Shared")
            pout = dram.tile([L, DM], dt.float32)
            pout_rs = dram.tile([L // num_cores, DM], dt.float32)

            # stage-limited pools (freed once early phases are emitted)
            ctxB = ExitStack()
            xpad_pool = ctxB.enter_context(tc.tile_pool(name="xpadp", bufs=1))
            xc_pool = ctxB.enter_context(tc.tile_pool(name="xcp", bufs=4))
            cacc_pool = ctxB.enter_context(tc.tile_pool(name="caccp", bufs=3))
            xev_pool = ctxB.enter_context(tc.tile_pool(name="xevp", bufs=2))
            et_pool = ctxB.enter_context(tc.tile_pool(name="etp", bufs=8))
            xdbl_pool = ctxB.enter_context(tc.tile_pool(name="xdblp", bufs=1))
            ctxA = ExitStack()
            hT_pool = ctxA.enter_context(tc.tile_pool(name="hTp", bufs=1))
            hnat_pool = ctxA.enter_context(tc.tile_pool(name="hnatp", bufs=4))
            w_in_pool = ctxA.enter_context(tc.tile_pool(name="winp", bufs=1))

            # ------------------------------------------- stage 1: hT = hidden^T
            psum_tp = ctxA.enter_context(
                tc.tile_pool(name="psum_tp", bufs=3, space="PSUM"))
            hT = {(k, q): hT_pool.tile([P, 4 * P], dt.bfloat16,
                                       name=f"hT{k}_{q}", tag=f"hT{k}_{q}")
                  for k in range(NKB) for q in range(NTT // 4)}
            for q in range(NTT // 4):
                hn = []
                for j in range(4):
                    t = hnat_pool.tile([P, DM], dt.float32, name="hnat", tag="hnat")
                    nc.sync.dma_start(
                        t[:], hidden[(q * 4 + j) * P:(q * 4 + j + 1) * P, :])
                    hn.append(t)
                for kb in range(NKB):
                    pt = psum_tp.tile([P, 4 * P], dt.float32, name="tp", tag="tp")
                    for j in range(4):
                        nc.tensor.transpose(
                            pt[:, j * P:(j + 1) * P],
                            hn[j][:, kb * P:(kb + 1) * P],
                            ident_sb[:],
                        )
                    nc.scalar.copy(hT[kb, q][:], pt[:])

            # ------------------------------------------- stage 2: in_proj
            w_in_sb = [w_in_pool.tile([P, 2 * CH], dt.bfloat16, name=f"win{k}",
                                      tag=f"win{k}") for k in range(NKB)]
            for k in range(NKB):
                nc.sync.dma_start(w_in_sb[k][:], w_inT[k * P:(k + 1) * P, :])

            conv_w_sb = {}
            conv_b_sb = {}
            dt_b_sb = {}
            A_sb = {}
            D_sb = {}
            for d in ("f", "r"):
                for pb in range(NPT):
                    ps = slice(pb * P, (pb + 1) * P)
                    for nm, store, src, shape in (
                        ("cw", conv_w_sb, conv_w, [P, KCONV]),
                        ("cb", conv_b_sb, conv_b, [P, 1]),
                        ("db", dt_b_sb, dt_b, [P, 1]),
                        ("A", A_sb, A_in, [P, NST]),
                        ("Dc", D_sb, D_in, [P, 1]),
                    ):
                        t = consts.tile(shape, dt.float32, name=f"{nm}{d}{pb}",
                                        tag=f"{nm}{d}{pb}")
                        nc.sync.dma_start(t[:], src[d][ps, :])
                        store[d, pb] = t
            w_dt_sb = {}
            for d in ("f", "r"):
                w_dt_sb[d] = consts.tile([RK, CH], dt.bfloat16, name=f"wdt{d}",
                                         tag=f"wdt{d}")
                nc.sync.dma_start(w_dt_sb[d][:], w_dtT[d][:])
            w_x_sb = {}
            for d in ("f", "r"):
                for pb in range(NPT):
                    t = consts.tile([P, NXP], dt.bfloat16, name=f"wx{d}{pb}",
                                    tag=f"wx{d}{pb}")
                    nc.sync.dma_start(t[:], w_xT[d][pb * P:(pb + 1) * P, :])
                    w_x_sb[d, pb] = t
            w_out_sb = []
            for pb in range(NPT):
                t = consts.tile([P, DM], dt.bfloat16, name=f"wo{pb}", tag=f"wo{pb}")
                nc.sync.dma_start(t[:], w_outT[pb * P:(pb + 1) * P, :])
                w_out_sb.append(t)


            xpad = [xpad_pool.tile([P, WPAD], dt.bfloat16, name=f"xpad{pb}",
                                   tag=f"xpad{pb}") for pb in range(NPT)]
            for pb in range(NPT):
                nc.vector.memset(xpad[pb][:, 0:PAD], 0.0)
                nc.vector.memset(xpad[pb][:, PAD + L:WPAD], 0.0)

            # x half first (feeds conv -> x_proj -> AllReduce critical path)
            for fb in range(NFB):
                for mb in range(NPT):
                    pm = psum_mm.tile([P, FB], dt.float32, name="mm", tag="mm")
                    for k in range(NKB):
                        nc.tensor.matmul(
                            pm[:],
                            w_in_sb[k][:, mb * P:(mb + 1) * P],
                            hT[k, fb][:],
                            start=(k == 0),
                            stop=(k == NKB - 1),
                        )
                    nc.scalar.copy(
                        xpad[mb][:, PAD + fb * FB: PAD + (fb + 1) * FB], pm[:])

            xc = {}

            def conv_block(d):
                """causal depthwise conv + silu."""
                for pb in range(NPT):
                    cw = conv_w_sb[d, pb]
                    cb = conv_b_sb[d, pb]
                    if d == "f":
                        taps = [xpad[pb][:, k:k + L] for k in range(KCONV)]
                    else:
                        taps = [xpad[pb][:, 2 * PAD - k: 2 * PAD - k + L][:, ::-1]
                                for k in range(KCONV)]
                    acc = cacc_pool.tile([P, L], dt.bfloat16, name="cacc", tag="cacc")
                    nc.scalar.activation(acc[:], taps[0], AF.Identity,
                                         bias=cb[:, 0:1], scale=cw[:, 0:1])
                    for k in range(1, KCONV):
                        acc2 = cacc_pool.tile([P, L], dt.bfloat16, name="cacc",
                                              tag="cacc")
                        nc.vector.scalar_tensor_tensor(
                            acc2[:], taps[k], cw[:, k:k + 1], acc[:],
                            OP.mult, OP.add)
                        acc = acc2
                    t = xc_pool.tile([P, L], dt.bfloat16, name="xc", tag="xc")
                    nc.scalar.activation(t[:], acc[:], AF.Silu)
                    xc[d, pb] = t

            def xproj_block(d, di):
                for fb in range(NFB):
                    pm = psum_mm.tile([NXP, FB], dt.float32, name="mmx", tag="mm")
                    for pb in range(NPT):
                        nc.tensor.matmul(
                            pm[:],
                            w_x_sb[d, pb][:],
                            xc[d, pb][:, fb * FB:(fb + 1) * FB],
                            start=(pb == 0),
                            stop=(pb == NPT - 1),
                        )
                    xev = xev_pool.tile([NXP, FB], dt.bfloat16, name="xev",
                                        tag="xev")
                    nc.scalar.copy(xev[:], pm[:])
                    nc.sync.dma_start(
                        xdbl_part[di * NXP:(di + 1) * NXP, fb * FB:(fb + 1) * FB],
                        xev[:])

            for di, d in enumerate(("f", "r")):
                conv_block(d)
                xproj_block(d, di)

            # AllReduce ASAP (bf16 payload)
            nc.gpsimd.collective_compute(
                "AllReduce",
                OP.add,
                replica_groups=[list(range(num_cores))],
                ins=[xdbl_part[:].opt()],
                outs=[xdbl_sum[:].opt()],
            )

            # ------------------- z half of in_proj + silu gating (overlaps AR)
            for mb in range(NPT, 2 * NPT):
                for fb in range(NFB):
                    pm = psum_mm.tile([P, FB], dt.float32, name="mm", tag="mm")
                    for k in range(NKB):
                        nc.tensor.matmul(
                            pm[:],
                            w_in_sb[k][:, mb * P:(mb + 1) * P],
                            hT[k, fb][:],
                            start=(k == 0),
                            stop=(k == NKB - 1),
                        )
                    pb = mb - NPT
                    nc.scalar.activation(
                        gz["f", pb][:, fb * FB:(fb + 1) * FB], pm[:], AF.Silu)
                    grev = gz["r", pb][:, ::-1]
                    nc.scalar.activation(
                        grev[:, fb * FB:(fb + 1) * FB], pm[:], AF.Silu)

            ctxA.close()

            dt_sb = {}
            dtx = {}
            dxc = {}
            xdbl = {}
            y = {}

            def dt_block(d):
                """Load x_dbl dt-rows, dt_proj + softplus, dtx, dxc."""
                xdbl[d] = xdbl_pool.tile([RK, L], dt.bfloat16, name="xdbl",
                                         tag=f"xdbl{d}")
                di = 0 if d == "f" else 1
                nc.sync.dma_start(xdbl[d][:],
                                  xdbl_sum[di * NXP:di * NXP + RK, :])
                # dt_proj matmuls + batched Exp then batched Ln (avoid ACT
                # table thrash)
                ets = {}
                for pb in range(NPT):
                    for fb in range(NFB):
                        pm = psum_mm.tile([P, FB], dt.float32, name="mm",
                                          tag="mm")
                        nc.tensor.matmul(
                            pm[:],
                            w_dt_sb[d][:, pb * P:(pb + 1) * P],
                            xdbl[d][0:RK, fb * FB:(fb + 1) * FB],
                            start=True, stop=True)
                        et = et_pool.tile([P, FB], dt.float32, name="etmp",
                                          tag="etmp")
                        nc.scalar.activation(
                            et[:], pm[:], AF.Exp, bias=dt_b_sb[d, pb][:, 0:1])
                        ets[pb, fb] = et
                for pb in range(NPT):
                    t = dt_pool.tile([P, L], dt.bfloat16, name="dtt",
                                     tag=f"dtt{d}{pb}")
                    for fb in range(NFB):
                        nc.scalar.activation(
                            t[:, fb * FB:(fb + 1) * FB], ets[pb, fb][:],
                            AF.Ln, bias=1.0)
                    dt_sb[d, pb] = t
                    tx = dtx_pool.tile([P, L], dt.bfloat16, name="dtx", tag="dtx")
                    nc.vector.tensor_mul(tx[:], dt_sb[d, pb][:], xc[d, pb][:])
                    dtx[d, pb] = tx
                    dc = dxc_pool.tile([P, L], dt.bfloat16, name="dxc", tag="dxc")
                    nc.vector.tensor_scalar_mul(
                        dc[:], xc[d, pb][:], D_sb[d, pb][:, 0:1])
                    dxc[d, pb] = dc

            def scan_block(d):
                """Selective scan for one direction; writes y[d, pb].

                Reduction over the 16 states runs as two accumulator chains
                per ptile: DVE chain (seeded with dxc) and GpSimd chain.
                GPS_N states go to the gpsimd chain.
                """
                if skip_scan:
                    for pb in range(NPT):
                        yt = y_pool.tile([P, L], dt.bfloat16, name="y", tag="y")
                        nc.vector.tensor_mul(yt[:], dtx[d, pb][:], gz[d, pb][:])
                        y[d, pb] = yt
                    return
                GPS_CHAIN = ()  # gpsimd stays dbx-only: its queue must never wait on DVE
                di = 0 if d == "f" else 1
                accv = {}
                accg = {}
                for n in range(NST):
                    rb = xdbl_sum[di * NXP + RK + n: di * NXP + RK + n + 1, :]
                    rc = xdbl_sum[di * NXP + RK + NST + n:
                                  di * NXP + RK + NST + n + 1, :]
                    bb = bbc_pool.tile([P, L], dt.bfloat16, name="bbc", tag="bbc")
                    nc.sync.dma_start(
                        bb[:], bass.AP(rb.tensor, rb.offset, [[0, P], [1, L]]))
                    cbt = cbc_pool.tile([P, L], dt.bfloat16, name="cbc", tag="cbc")
                    nc.sync.dma_start(
                        cbt[:], bass.AP(rc.tensor, rc.offset, [[0, P], [1, L]]))
                    for pb in range(NPT):
                        da = da_pool.tile([P, L], dt.float32, name="da", tag="da")
                        nc.scalar.activation(
                            da[:], dt_sb[d, pb][:], AF.Exp,
                            scale=A_sb[d, pb][:, n:n + 1])
                        dbx = dbx_pool.tile([P, L], dt.bfloat16, name="dbx",
                                            tag="dbx")
                        nc.gpsimd.tensor_tensor(dbx[:], dtx[d, pb][:], bb[:],
                                                OP.mult)
                        h = h_pool.tile([P, L], dt.bfloat16, name="h", tag="h")
                        nc.vector.tensor_tensor_scan(
                            h[:], da[:], dbx[:], 0.0, OP.mult, OP.add)
                        # hC product; seeds are written straight into a chain
                        # buffer so the pool ping-pong never clobbers a live
                        # accumulator.
                        gps_n = n in GPS_CHAIN
                        if gps_n:
                            pool_, tag_, acc_ = accg_pool, f"accg{pb}", accg
                        else:
                            pool_, tag_, acc_ = accv_pool, f"accv{pb}", accv
                        if (d, pb) not in acc_:
                            hc = pool_.tile([P, L], dt.bfloat16, name="hc",
                                            tag=tag_)
                            nc.vector.tensor_mul(hc[:], h[:], cbt[:])
                            acc_[d, pb] = hc
                        else:
                            hc = hc_pool.tile([P, L], dt.bfloat16, name="hc",
                                              tag="hc")
                            nc.vector.tensor_mul(hc[:], h[:], cbt[:])
                            s = pool_.tile([P, L], dt.bfloat16, name="acc",
                                           tag=tag_)
                            if gps_n:
                                nc.gpsimd.tensor_tensor(
                                    s[:], acc_[d, pb][:], hc[:], OP.add)
                            else:
                                nc.vector.tensor_add(
                                    s[:], acc_[d, pb][:], hc[:])
                            acc_[d, pb] = s
                for pb in range(NPT):
                    # merge chains + D*x + gate
                    m = accv[d, pb]
                    if (d, pb) in accg:
                        m2 = accv_pool.tile([P, L], dt.bfloat16, name="accm",
                                            tag=f"accv{pb}")
                        nc.vector.tensor_add(m2[:], m[:], accg[d, pb][:])
                        m = m2
                    m2 = accg_pool.tile([P, L], dt.bfloat16, name="accm2",
                                        tag=f"accg{pb}")
                    nc.vector.tensor_add(m2[:], m[:], dxc[d, pb][:])
                    yt = y_pool.tile([P, L], dt.bfloat16, name="y", tag="y")
                    nc.vector.tensor_mul(yt[:], m2[:], gz[d, pb][:])
                    y[d, pb] = yt

            dt_block("f")
            dt_block("r")
            ctxB.close()

            # ------------------------------------------- scan pools
            bbc_pool = ctx.enter_context(tc.tile_pool(name="bbcp", bufs=3))
            cbc_pool = ctx.enter_context(tc.tile_pool(name="cbcp", bufs=3))
            da_pool = ctx.enter_context(tc.tile_pool(name="dap", bufs=2))
            dbx_pool = ctx.enter_context(tc.tile_pool(name="dbxp", bufs=4))
            h_pool = ctx.enter_context(tc.tile_pool(name="hp", bufs=2))
            hc_pool = ctx.enter_context(tc.tile_pool(name="hcp", bufs=3))
            accv_pool = ctx.enter_context(tc.tile_pool(name="accvp", bufs=2))
            accg_pool = ctx.enter_context(tc.tile_pool(name="accgp", bufs=2))
            y_pool = ctx.enter_context(tc.tile_pool(name="yp", bufs=4))

            scan_block("f")
            scan_block("r")

            # ------------------------------------------- out_proj + RS
            for rs in range(NRS):
                for tbo in range(RSROWS // P):
                    tb = rs * (RSROWS // P) + tbo
                    ycb = {}
                    for pb in range(NPT):
                        yct = yc_pool.tile([P, P], dt.bfloat16, name="ycb",
                                           tag="ycb")
                        nc.vector.tensor_add(
                            yct[:], y["f", pb][:, tb * P:(tb + 1) * P],
                            y["r", pb][:, ::-1][:, tb * P:(tb + 1) * P])
                        ycb[pb] = yct
                    for fb in range(DM // FB):
                        pm = psum_mm.tile([P, FB], dt.float32, name="mm",
                                          tag="mm")
                        for pb in range(NPT):
                            nc.tensor.matmul(
                                pm[:],
                                ycb[pb][:],
                                w_out_sb[pb][:, fb * FB:(fb + 1) * FB],
                                start=(pb == 0),
                                stop=(pb == NPT - 1),
                            )
                        oev = oev_pool.tile([P, FB], dt.float32, name="oev",
                                            tag="oev")
                        nc.scalar.copy(oev[:], pm[:])
                        nc.sync.dma_start(
                            pout[tb * P:(tb + 1) * P, fb * FB:(fb + 1) * FB],
                            oev[:])
                # chunk RS as soon as its token rows are written
                nc.gpsimd.collective_compute(
                    "ReduceScatter",
                    OP.add,
                    replica_groups=[list(range(num_cores))],
                    ins=[pout[rs * RSROWS:(rs + 1) * RSROWS, :].opt()],
                    outs=[pout_rs[rs * (RSROWS // num_cores):
                                  (rs + 1) * (RSROWS // num_cores), :].opt()],
                )
                nc.sync.dma_start(
                    out[rs * (RSROWS // num_cores):
                        (rs + 1) * (RSROWS // num_cores), :],
                    pout_rs[rs * (RSROWS // num_cores):
                            (rs + 1) * (RSROWS // num_cores), :])

    return nc


# ---------------------------------------------------------------- host side
def _make_in_maps(inputs):
    """Slice/transpose the full inputs into per-core input dicts."""
    h = np.ascontiguousarray(np.asarray(inputs["hidden_states"],
                                        dtype=np.float32).reshape(L, DM))
    w_in = np.asarray(inputs["in_proj_w"], dtype=np.float32)     # (2DI, DM)
    w_out = np.asarray(inputs["out_proj_w"], dtype=np.float32)   # (DM, DI)
    ident = np.eye(P, dtype=np.float32)

    in_maps = []
    for c in range(NCORES):
        sl = slice(c * CH, (c + 1) * CH)
        m = {"hidden": h, "ident": ident}
        w_slice = np.concatenate(
            [w_in[sl, :], w_in[DI + c * CH: DI + (c + 1) * CH, :]], axis=0)
        m["w_inT"] = np.ascontiguousarray(
            w_slice.T).astype(ml_dtypes.bfloat16)                 # (DM, 2CH)
        m["w_outT"] = np.ascontiguousarray(
            w_out[:, sl].T).astype(ml_dtypes.bfloat16)            # (CH, DM)
        for d, tag in (("f", "_f"), ("r", "_r")):
            w_x = np.asarray(inputs[f"x_proj_w{tag}"], dtype=np.float32)
            m[f"w_xT_{d}"] = np.ascontiguousarray(
                w_x[:, sl].T).astype(ml_dtypes.bfloat16)          # (CH, 96)
            w_dt = np.asarray(inputs[f"dt_proj_w{tag}"], dtype=np.float32)
            m[f"w_dtT_{d}"] = np.ascontiguousarray(
                w_dt[sl, :].T).astype(ml_dtypes.bfloat16)         # (RK, CH)
            m[f"conv_w_{d}"] = np.ascontiguousarray(
                np.asarray(inputs[f"conv_w{tag}"], dtype=np.float32)[sl, :])
            m[f"conv_b_{d}"] = np.ascontiguousarray(
                np.asarray(inputs[f"conv_b{tag}"], dtype=np.float32)[sl, None])
            m[f"dt_b_{d}"] = np.ascontiguousarray(
                np.asarray(inputs[f"dt_proj_b{tag}"], dtype=np.float32)[sl, None])
            m[f"A_{d}"] = np.ascontiguousarray(
                -np.exp(np.asarray(inputs[f"A_log{tag}"], dtype=np.float32)[sl, :]))
            m[f"D_{d}"] = np.ascontiguousarray(
                np.asarray(inputs[f"D{tag}"], dtype=np.float32)[sl, None])
        in_maps.append(m)
    return in_maps


_CACHED = {}


def _install_ntff_hook_shim():
    """The agent image's antenv lacks axon_hooks; provide it and register
    the ctypes-based NTFF profile hook from trn_agent_boot."""
    import types
    try:
        import antenv.axon_hooks  # noqa: F401
        return
    except ImportError:
        pass
    import antenv
    mod = types.ModuleType("antenv.axon_hooks")
    _state = {"h": None}
    mod.get_axon_ntff_profile_hook = lambda: _state["h"]
    mod.set_axon_ntff_profile_hook = lambda h: _state.__setitem__("h", h)
    sys.modules["antenv.axon_hooks"] = mod
    antenv.axon_hooks = mod
    try:
        from trn_agent_boot.trn_boot import _ntff_profile_via_ctypes
        hook = _ntff_profile_via_ctypes("/opt/axon/libaxon_pjrt.so")
        if hook is not None:
            mod.set_axon_ntff_profile_hook(hook)
    except Exception:
        pass


def _install_hook_err_capture():
    """Wrap the neuronx_cc hook so compile errors land in hook_err.log
    instead of being swallowed by the PJRT boundary."""
    import traceback
    import concourse.bass2jax as b2j
    if getattr(b2j, "_err_capture_installed", False):
        return
    orig = b2j.neuronx_cc_hook

    def wrapped(*a):
        try:
            return orig(*a)
        except Exception:
            with open("/tmp/hook_err.log", "w") as f:
                f.write(traceback.format_exc())
            raise

    b2j.neuronx_cc_hook = wrapped
    b2j._err_capture_installed = True


def kernel(**inputs):
    from concourse.bass_utils import run_bass_kernel_spmd

    _install_ntff_hook_shim()
    _install_hook_err_capture()

    if "nc" not in _CACHED:
        from concourse.bass_interp import get_hw_module
        nc = build_program(
            skip_scan=bool(int(os.environ.get("KERNEL_SKIP_SCAN", "0"))))
        nc.finalize()  # bacc: register allocation, library/ACT-table loads
        nc.m = get_hw_module(nc.m)  # strip sim-only callback instructions
        _CACHED["nc"] = nc
    nc = _CACHED["nc"]

    in_maps = _make_in_maps(inputs)
    res = run_bass_kernel_spmd(
        nc, in_maps, core_ids=list(range(NCORES)),
        trace=bool(int(os.environ.get("KERNEL_TRACE", "0"))),
    )
    _CACHED["last_result"] = res
    # Chunked ReduceScatter permutes row ownership: core c's out rows are
    # [rs*RSROWS + c*(RSROWS/8) : +RSROWS/8) for each rs chunk.
    rows = RSROWS // NCORES
    full = np.empty((L, DM), dtype=np.float32)
    for c in range(NCORES):
        o = res.results[c]["out"]
        for rs in range(NRS):
            full[rs * RSROWS + c * rows: rs * RSROWS + (c + 1) * rows, :] = \
                o[rs * rows:(rs + 1) * rows, :]
    return full.reshape(1, L, DM)


if __name__ == "__main__":
    nc = build_program()
    try:
        n = sum(len(bb.instructions) for bb in nc.main_func.blocks)
    except Exception:
        n = "?"
    print("build ok; instructions:", n)
